# revision 5
# baseline (speedup 1.0000x reference)
"""Trainium2 Bass kernel: 4096x4096 valid 5x5 cross-correlation + scalar bias.

Strategy (8 NeuronCores, SPMD):
  - Shard the OUTPUT by columns: core c computes out[:, 512c : 512c+512]
    (core 7's last 4 columns are padding, trimmed after gather). Each core
    reads x rows 0..4095, cols [512c, 512c+516) (host-padded to width 4100).
  - On-core: the 5x5 conv is computed as banded-matrix matmuls on the
    TensorEngine. For an input row-tile X_g = x[124g : 124g+128, :] and
    kernel column dj, the banded matrix B_dj[k, m] = w[k-m, dj] gives
      (B_dj^T @ X_g[:, dj:dj+512])[m, n] = sum_di w[di, dj] x[124g+m+di, n+dj]
    so accumulating the 5 dj-matmuls in PSUM yields 124 valid output rows
    per tile. 4092 = 33 * 124 exactly.
  - HOST-PACKED I/O LAYOUT: x is pre-gathered on the host into a
    [128, 33*516] bf16 array whose partition p, segment g holds
    x[124g+p, :]. Input streams in 6 large dma_starts (128 descriptors
    each, multi-KB contiguous per partition) — the DMA path is
    descriptor-rate-bound (~9.4ns/desc/queue), so fewer+bigger pushes win.
  - Output is bf16 (halves HBM write traffic AND descriptor bytes; the
    fp32 version saturated the shared DMA engines mid-stream and left a
    14us output-backlog tail). Host converts to fp32 after the gather.
  - PSUM: one 4-bank mega-tile per 4-group block; the whole block drains
    with a SINGLE DVE tensor_scalar_add (fused bias, fp32->bf16) instead
    of 4 per-group drains. Fewer instructions -> fewer Tile semaphores ->
    shorter kernel-tail sem-reset storm (the Tile teardown clears every
    allocated sem at ~90ns apiece per engine).
  - WARM-UP: the Tensor engine ramps 0.65->1.2->2.4 GHz over ~4.5us of
    sustained activity. Dummy matmuls on a memset tile bridge the window
    between tensor-sequencer-ready (~7.3us preamble) and first-chunk-
    consumable (~10us) so the real stream runs at full clock.
"""
import os

os.environ.setdefault("MYCRO_LOCAL_CACHE", "1")

import numpy as np

import concourse.bass as bass
import concourse.bacc as bacc
import concourse.tile as tile
import concourse.mybir as mybir
from concourse import bass_utils

H, W = 4096, 4096
KH, KW = 5, 5
OH, OW = H - KH + 1, W - KW + 1          # 4092, 4092
NCORES = 8
COLS = 512                               # output cols per core
XC = COLS + KW - 1                       # 516 input cols per core
NG = 33                                  # row tiles per core (33*124 = 4092)
RV = 124                                 # valid output rows per tile

# Input DMA granularity: few, large pushes (descriptor-rate-bound path).
# First pushes small so the first matmul starts early.
PUSHES = [1, 2, 4, 8, 8, 10]
assert sum(PUSHES) == NG
# Matmul/drain/output-store granularity. One 4-bank PSUM mega-tile per
# block; two blocks in flight (accumulate + drain) = 8 banks. Tail blocks
# small so the final drain+DMA chain after the last matmul is short.
BLOCKS = [1, 2, 4, 4, 4, 4, 4, 4, 4, 1, 1]
assert sum(BLOCKS) == NG and max(BLOCKS) * 2 * 512 * 4 <= 8 * 2048

WARM_MM = 12
WARM_ROWS = 256
BT = KW * 128                             # banded-weight cols at xs[:, 0:BT]

_compiled = None
TRACE = False            # test harness can flip this for neuron-profile timing
LAST_EXEC_NS = None


def _build():
    nc = bacc.Bacc("TRN2", target_bir_lowering=False, debug=False,
                   num_devices=NCORES)
    mdt = mybir.dt.bfloat16

    # xs = [banded weights | packed x]: one dtype, so the first dma_start
    # delivers the weights AND the first row-group together.
    x_dram = nc.dram_tensor("xs", (128, BT + NG * XC), mdt,
                            kind="ExternalInput")
    bias_dram = nc.dram_tensor("biast", (128, 1), mybir.dt.float32,
                               kind="ExternalInput")
    # NOTE: full 128 partitions on purpose — a 124-row DMA falls off the
    # HWDGE fast path (DIRECT2D descgen takes ~8us per push vs ~0.3us).
    out_dram = nc.dram_tensor("out", (128, NG * COLS), mdt,
                              kind="ExternalOutput")

    # group g -> (push index, local group offset within push)
    gmap = []
    for p, pk in enumerate(PUSHES):
        for lg in range(pk):
            gmap.append((p, lg))

    with tile.TileContext(nc) as tc:
        with (
            tc.tile_pool(name="const", bufs=1) as cpool,
            tc.tile_pool(name="x", bufs=len(PUSHES)) as xpool,
            tc.tile_pool(name="stage", bufs=4) as spool,
            tc.tile_pool(name="psum", bufs=2, space=bass.MemorySpace.PSUM) as ppool,
        ):
            # PE warm-up: memset on GpSimd (idle at kernel start), then
            # dummy matmuls into a PSUM tile that is never read.
            warm = cpool.tile([128, COLS], mdt)
            nc.gpsimd.memset(warm[:], 0.0)
            wps = ppool.tile([128, COLS], mybir.dt.float32, name="warmps",
                             tag="ps")
            for i in range(WARM_MM):
                nc.tensor.matmul(wps[:, 0:WARM_ROWS], warm[:, 0:128],
                                 warm[:, 0:WARM_ROWS],
                                 start=True, stop=True)

            biast = cpool.tile([128, 1], mybir.dt.float32)
            bt = cpool.tile([128, BT], mdt)
            # Parallel critical-path loads: weights via sync ring, push 0
            # via the (otherwise idle until the first drain) scalar ring —
            # the two ~0.7us descgens and transfers overlap.
            nc.sync.dma_start(bt[:], x_dram.ap()[:, 0:BT])
            xts, off = [], 0
            for k, pk in enumerate(PUSHES):
                xt = xpool.tile([128, pk * XC], mdt, tag="x")
                ring = nc.scalar if k == 0 else nc.sync
                ring.dma_start(
                    xt[:],
                    x_dram.ap()[:, BT + off * XC:BT + (off + pk) * XC])
                xts.append(xt)
                off += pk
                if k == 0:
                    nc.scalar.dma_start(biast[:], bias_dram.ap())

            g0 = 0
            for bi, bk in enumerate(BLOCKS):
                ps = ppool.tile([128, bk * COLS], mybir.dt.float32,
                                name=f"ps{bi}", tag="ps")
                # weight-stationary sweep: dj outer, groups inner
                for dj in range(KW):
                    for gl in range(bk):
                        p, lg = gmap[g0 + gl]
                        nc.tensor.matmul(
                            ps[:, gl * COLS:(gl + 1) * COLS],
                            bt[:, dj * 128:(dj + 1) * 128],
                            xts[p][:, lg * XC + dj:lg * XC + dj + COLS],
                            start=(dj == 0),
                            stop=(dj == KW - 1),
                        )
                stg = spool.tile([128, bk * COLS], mdt)
                last = bi == len(BLOCKS) - 1
                if last:
                    # tail block (1 group): halve the critical chain by
                    # draining 256-col halves on DVE and ACT in parallel,
                    # then writing each half on its own ring
                    hw_ = COLS // 2
                    nc.vector.tensor_scalar_add(stg[:, 0:hw_],
                                                ps[:, 0:hw_], biast[:])
                    nc.scalar.activation(stg[:, hw_:COLS],
                                         ps[:, hw_:COLS],
                                         mybir.ActivationFunctionType.Identity,
                                         bias=biast[:])
                    nc.sync.dma_start(
                        out_dram.ap()[:, g0 * COLS:g0 * COLS + hw_],
                        stg[:, 0:hw_])
                    nc.scalar.dma_start(
                        out_dram.ap()[:, g0 * COLS + hw_:(g0 + 1) * COLS],
                        stg[:, hw_:COLS])
                    g0 += bk
                    continue
                # single-instruction block drain on DVE with fused bias
                # (fp32 PSUM -> bf16 SBUF); one output push per block,
                # alternating rings so descgen + completion work is split.
                nc.vector.tensor_scalar_add(stg[:], ps[:], biast[:])
                ring = nc.scalar if bi % 2 == 0 else nc.sync
                ring.dma_start(
                    out_dram.ap()[:, g0 * COLS:(g0 + bk) * COLS], stg[:])
                g0 += bk

    nc.compile()
    return nc


def _banded(weight: np.ndarray) -> np.ndarray:
    ball = np.zeros((128, KW * 128), dtype=np.float32)
    for dj in range(KW):
        for di in range(KH):
            m = np.arange(128 - di)
            ball[m + di, dj * 128 + m] = weight[di, dj]
    return ball


def kernel(x: np.ndarray, weight: np.ndarray, bias: np.ndarray) -> np.ndarray:
    global _compiled
    import ml_dtypes
    bf16 = ml_dtypes.bfloat16

    x = np.asarray(x, dtype=np.float32)
    weight = np.asarray(weight, dtype=np.float32)
    bias = np.asarray(bias, dtype=np.float32)

    if _compiled is None:
        _compiled = _build()
    nc = _compiled

    xpad = np.zeros((H, NCORES * COLS + KW - 1), dtype=bf16)
    xpad[:, :W] = x.astype(bf16)
    ball = _banded(weight).astype(bf16)
    bias_col = np.full((128, 1), bias[0], dtype=np.float32)

    # pack: xs = [banded weights | xp], xp[p, g*XC + c] = x[124g+p, 512c0+c]
    idx = (124 * np.arange(NG)[:, None] + np.arange(128)[None, :])  # (NG,128)
    in_maps = []
    for c in range(NCORES):
        xc = xpad[:, COLS * c: COLS * c + XC]      # (4096, XC) view
        xp = xc[idx, :]                            # (NG, 128, XC)
        xs = np.empty((128, BT + NG * XC), dtype=bf16)
        xs[:, :BT] = ball
        xs[:, BT:] = xp.transpose(1, 0, 2).reshape(128, NG * XC)
        in_maps.append({"xs": xs, "biast": bias_col})

    res = bass_utils.run_bass_kernel_spmd(nc, in_maps,
                                          core_ids=list(range(NCORES)),
                                          trace=TRACE)
    global LAST_EXEC_NS
    LAST_EXEC_NS = res.exec_time_ns

    # unpack: out[124g + m, 512c + n] = op[m, g*COLS + n]  (m < 124)
    cols = []
    for c in range(NCORES):
        op = np.asarray(res.results[c]["out"],
                        dtype=np.float32).reshape(128, NG, COLS)
        cols.append(op[:RV].transpose(1, 0, 2).reshape(OH, COLS))
    out = np.hstack(cols)
    return np.ascontiguousarray(out[:, :OW])
